# revision 27
# baseline (speedup 1.0000x reference)
"""3-layer LIF spiking network on Trainium2 via Bass/Tile.

kernel(**inputs) takes FULL unsharded numpy inputs, returns FULL (B, T, O)
output.  Data-parallel: batch 1024 -> 8 cores x 128.  Weights replicated.

Device algorithm (per core, batch 128 on free axis):
  - Combined L1+L2 state tile (128 partitions = [64 h1 | 64 h2], 128 batch),
    L2 skewed one step behind L1 so spikes s1(t) feed cur2(t) via the PE.
  - Per step: PE accumulates psum = [c1(t) - s1(t-1); W2@s1(t-1) - s2(t-2)]
    (x-pass with W1T + single combined A-pass), DVE does
    m = m*beta + psum (scalar_tensor_tensor) and s = (m > theta) (is_gt),
    writing spikes into an SBUF history.  Biases are folded into per-unit
    thresholds by the shift m~ = m - b/(1-beta) when safe.
  - Layer 3 (pure leaky integrator, no reset) leaves the recurrence:
    in-loop matmul c3(t) = s2(t)^T-row-major @ W3T into (batch, 20) psum,
    ACT evacuates to SBUF, and a post-loop tensor_tensor_scan along time
    computes m3 for all t in 20 instructions.
Matmul operands are bf16 (PSUM accumulation fp32); membrane state fp32.
"""

import numpy as np
import ml_dtypes
from contextlib import ExitStack

B_FULL, T, D_IN, H, O = 1024, 512, 128, 64, 20
N_CORES = 8
BL = B_FULL // N_CORES  # 128 batch per core
P = 128
THRESH = 1.0

_BF16 = ml_dtypes.bfloat16

_module_cache = {}


def _to_bf16(a):
    """Fast vectorized fp32 -> bf16 with round-to-nearest-even."""
    a = np.ascontiguousarray(a, np.float32)
    u = a.view(np.uint32)
    r = (u >> 16) & np.uint32(1)
    out = ((u + np.uint32(0x7FFF) + r) >> 16).astype(np.uint16)
    return out.view(_BF16)


def _build_module(use_shift: bool, repeat: int = 1):
    import concourse.bass as bass
    import concourse.tile as tile
    from concourse import bacc, mybir

    f32 = mybir.dt.float32
    bf16 = mybir.dt.bfloat16
    Alu = mybir.AluOpType

    nc = bacc.Bacc("TRN2", target_bir_lowering=False, debug=False,
                   num_devices=N_CORES)

    xT_d = nc.dram_tensor("xT", [P, T, BL], bf16, kind="ExternalInput").ap()
    A_d = nc.dram_tensor("A", [P, P], bf16, kind="ExternalInput").ap()
    W1T_d = nc.dram_tensor("W1T", [P, P], bf16, kind="ExternalInput").ap()
    W3T_d = nc.dram_tensor("W3T", [H, O], bf16, kind="ExternalInput").ap()
    bt_d = nc.dram_tensor("bt", [P, 1], f32, kind="ExternalInput").ap()
    th_d = nc.dram_tensor("th", [P, 1], f32, kind="ExternalInput").ap()
    m0_d = nc.dram_tensor("m0", [P, BL], f32, kind="ExternalInput").ap()
    bt3_d = nc.dram_tensor("bt3b", [P, O], f32, kind="ExternalInput").ap()
    b3b_d = nc.dram_tensor("b3b", [P, O], f32, kind="ExternalInput").ap()
    b12_d = nc.dram_tensor("b12", [1, P], bf16, kind="ExternalInput").ap()
    V_d = nc.dram_tensor("V", [BL, T, O], f32, kind="ExternalOutput").ap()

    CH = 16   # x prefetch chunk (timesteps)
    CHT = 128  # layer-3 scan/output chunk (timesteps)

    with ExitStack() as ctx:
        tc = ctx.enter_context(tile.TileContext(nc))
        const = ctx.enter_context(tc.tile_pool(name="const", bufs=1))
        big = ctx.enter_context(tc.tile_pool(name="big", bufs=1))
        xpool = ctx.enter_context(tc.tile_pool(name="xc", bufs=3))
        ps12 = ctx.enter_context(tc.tile_pool(name="ps12", bufs=5, space="PSUM"))
        ps3 = ctx.enter_context(tc.tile_pool(name="ps3", bufs=3, space="PSUM"))

        A_sb = const.tile([P, P], bf16)
        nc.sync.dma_start(A_sb[:], A_d)
        W1T_sb = const.tile([P, P], bf16)
        nc.sync.dma_start(W1T_sb[:], W1T_d)
        W3T_sb = const.tile([P, O], bf16)
        nc.sync.dma_start(W3T_sb[H:P, :], W3T_d)
        bt_sb = const.tile([P, 1], f32)
        nc.sync.dma_start(bt_sb[:], bt_d)
        th_sb = const.tile([P, 1], f32)
        nc.sync.dma_start(th_sb[:], th_d)
        m0_sb = const.tile([P, BL], f32)
        nc.sync.dma_start(m0_sb[:], m0_d)
        bt3_sb = const.tile([P, O], f32)
        nc.sync.dma_start(bt3_sb[:], bt3_d)
        b3b_sb = const.tile([P, O], f32)
        nc.sync.dma_start(b3b_sb[:], b3b_d)
        b12_sb = const.tile([1, P], bf16)
        nc.sync.dma_start(b12_sb[:], b12_d)

        ones_row = const.tile([1, BL], bf16)
        nc.vector.memset(ones_row[:], 1.0)

        s_hist = big.tile([P, T + 1, BL], bf16)
        nc.vector.memset(s_hist[:, 0, :], 0.0)
        s_last = const.tile([P, BL], bf16)
        c3 = big.tile([BL, T, O], f32)
        m_sb = const.tile([P, BL], bf16)

        xch = [None] * 3

        for _rep in range(repeat):
          nc.vector.tensor_copy(m_sb[:], m0_sb[:])
          pending = []
          for k in range(T + 1):
            if k < T:
                if k % CH == 0:
                    cki = (k // CH) % 3
                    xch[cki] = xpool.tile([P, CH, BL], bf16, name="xch",
                                          tag="xch")
                    nc.sync.dma_start(xch[cki][:], xT_d[:, k:k + CH, :])
                xt = xch[(k // CH) % 3][:, k % CH, :]

            pk = ps12.tile([P, BL], f32)
            first = True
            if not use_shift:
                nc.tensor.matmul(pk[:], b12_sb[:], ones_row[:],
                                 start=True, stop=False)
                first = False
            if k < T:
                nc.tensor.matmul(pk[:], W1T_sb[:], xt,
                                 start=first, stop=False)
                first = False
            nc.tensor.matmul(pk[:], A_sb[:], s_hist[:, k, :],
                             start=first, stop=True)

            nc.vector.scalar_tensor_tensor(
                m_sb[:], m_sb[:], bt_sb[:], pk[:],
                op0=Alu.mult, op1=Alu.add)
            s_dst = s_hist[:, k + 1, :] if k < T else s_last[:]
            nc.vector.tensor_scalar(
                s_dst, m_sb[:], th_sb[:], None, op0=Alu.is_gt)

            if k == 0:
                # L2 rows saw one spurious decay step; re-init state and
                # clear the (junk) s2 half of slot 1.
                nc.vector.tensor_copy(m_sb[H:P, :], m0_sb[H:P, :])
                nc.vector.tensor_scalar_mul(
                    s_hist[H:P, 1, :], s_hist[H:P, 1, :], 0.0)

            # L3 matmul for step k-2: same spike dependency as this
            # iteration's A-pass, but emitted AFTER it so it fills PE
            # idle time instead of delaying the recurrence-critical MM.
            if k >= 2:
                p3 = ps3.tile([BL, O], f32)
                nc.tensor.matmul(p3[:], s_hist[H:P, k, :], W3T_sb[H:P, :],
                                 start=True, stop=True)
                nc.scalar.copy(c3[:, k - 2, :], p3[:])

            # layer-3 leaky scan + output DMA for finished CHT-chunks,
            # spread one DVE op per few iterations so the recurrence
            # never stalls on a burst of scan work.
            if k > 0 and k % CHT == 0:
                pend_t0 = k - CHT
                pend = []
                for o in range(O):
                    pend.append(("scan", pend_t0, o))
                pend.append(("dma", pend_t0, 0))
                pend.append(("dma", pend_t0, 1))
                pending.append(pend)

            def emit_one():
                cur = pending[0]
                kind, t0, o = cur.pop(0)
                if kind == "scan":
                    v = c3[:, t0:t0 + CHT, o]
                    nc.gpsimd.tensor_scalar_add(v, v, b3b_sb[:, o:o + 1])
                    init = 0.0 if t0 == 0 else c3[:, t0 - 1, o:o + 1]
                    nc.vector.tensor_tensor_scan(
                        v, bt3_sb[:, o:o + 1].broadcast_to([BL, CHT]),
                        v, init, op0=Alu.mult, op1=Alu.add)
                else:
                    hc = CHT // 2
                    nc.sync.dma_start(
                        V_d[:, t0 + o * hc:t0 + (o + 1) * hc, :],
                        c3[:, t0 + o * hc:t0 + (o + 1) * hc, :])
                if not cur:
                    pending.pop(0)

            if pending and k % 5 == 0:
                emit_one()

          # final L3 step (T-1) from s_last, then drain remaining scan work
          p3f = ps3.tile([BL, O], f32, name="p3f", tag="p3")
          nc.tensor.matmul(p3f[:], s_last[H:P, :], W3T_sb[H:P, :],
                           start=True, stop=True)
          nc.scalar.copy(c3[:, T - 1, :], p3f[:])
          while pending:
            emit_one()

    nc.compile()
    return nc


def _get_module(use_shift: bool, repeat: int = 1):
    key = (use_shift, repeat)
    if key not in _module_cache:
        _module_cache[key] = _build_module(use_shift, repeat)
    return _module_cache[key]


def _prep_inputs(x, W1, b1, beta1, W2, b2, beta2, W3, b3, beta3):
    """Host-side prep -> (use_shift, list of per-core in_maps)."""
    x = np.ascontiguousarray(x, np.float32)
    bt1 = np.clip(np.asarray(beta1, np.float32), 0.0, 1.0)
    bt2 = np.clip(np.asarray(beta2, np.float32), 0.0, 1.0)
    bt3 = np.clip(np.asarray(beta3, np.float32), 0.0, 1.0)
    b1 = np.asarray(b1, np.float32)
    b2 = np.asarray(b2, np.float32)
    b3 = np.asarray(b3, np.float32)
    W1 = np.asarray(W1, np.float32)
    W2 = np.asarray(W2, np.float32)
    W3 = np.asarray(W3, np.float32)

    bt12 = np.concatenate([bt1, bt2])          # (128,)
    b12 = np.concatenate([b1, b2])             # (128,)
    with np.errstate(divide="ignore", invalid="ignore"):
        mu12 = b12.astype(np.float64) / (1.0 - bt12.astype(np.float64))
        mu3 = b3.astype(np.float64) / (1.0 - bt3.astype(np.float64))
    use_shift = bool(
        np.all(np.abs(1.0 - bt12) > 1e-3) and np.all(np.abs(mu12) < 50.0)
    )

    if use_shift:
        mu12 = mu12.astype(np.float32)
        theta = (THRESH - mu12).reshape(P, 1)
        m0 = np.broadcast_to((-mu12).reshape(P, 1), (P, BL)).copy()
        b12_t = np.zeros((1, P), np.float32)
    else:
        theta = np.full((P, 1), THRESH, np.float32)
        m0 = np.zeros((P, BL), np.float32)
        b12_t = b12.reshape(1, P)

    # A = [[-I, W2T], [0, -I]]
    A = np.zeros((P, P), np.float32)
    A[0:H, 0:H] = -np.eye(H, dtype=np.float32)
    A[0:H, H:P] = W2.T
    A[H:P, H:P] = -np.eye(H, dtype=np.float32)

    W1Tp = np.zeros((P, P), np.float32)
    W1Tp[:, 0:H] = W1.T

    shared = {
        "A": _to_bf16(A),
        "W1T": _to_bf16(W1Tp),
        "W3T": _to_bf16(W3.T),
        "bt": np.ascontiguousarray(bt12.reshape(P, 1), np.float32),
        "th": np.ascontiguousarray(theta, np.float32),
        "m0": np.ascontiguousarray(m0, np.float32),
        "bt3b": np.ascontiguousarray(
            np.broadcast_to(bt3, (P, O)), np.float32),
        "b3b": np.ascontiguousarray(np.broadcast_to(b3, (P, O)), np.float32),
        "b12": _to_bf16(b12_t),
    }

    # xT per core: (D, T, B_local) bf16 from x (B, T, D)
    xb = _to_bf16(x)                                 # (B, T, D)
    xb = xb.reshape(N_CORES, BL, T, D_IN)
    in_maps = []
    for c in range(N_CORES):
        xTc = np.ascontiguousarray(xb[c].transpose(2, 1, 0))  # (D, T, BL)
        m = dict(shared)
        m["xT"] = xTc
        in_maps.append(m)
    return use_shift, in_maps


def kernel(x, W1, b1, beta1, W2, b2, beta2, W3, b3, beta3):
    from concourse.bass_utils import run_bass_kernel_spmd

    use_shift, in_maps = _prep_inputs(
        x, W1, b1, beta1, W2, b2, beta2, W3, b3, beta3)
    nc = _get_module(use_shift)
    res = run_bass_kernel_spmd(nc, in_maps, core_ids=list(range(N_CORES)))
    out = np.empty((B_FULL, T, O), np.float32)
    for c in range(N_CORES):
        out[c * BL:(c + 1) * BL] = res.results[c]["V"]
    return out
